# revision 61
# baseline (speedup 1.0000x reference)
"""PointFlow2DVAE loss kernel for 8 Trainium2 NeuronCores.

Data-parallel over batch B=8: one point cloud per core. Each core runs the
PointNet encoder, the combined Euler CNF integration (forward + generation
branch) with the exact-divergence computation folded into precomputed
matmuls, and the chamfer distance. Cores emit small partial-sum vectors;
the final scalar combine happens on host.
"""
import numpy as np

import concourse.bacc as bacc
import concourse.bass as bass
import concourse.tile as tile
from concourse import mybir
from concourse.bass_utils import run_bass_kernel_spmd

B, N, D = 8, 2048, 2
LAT, ENC_H, CNF_H = 128, 256, 256
STEPS = 10
DT = 1.0 / STEPS
LAM_R, LAM_P, LAM_E, LAM_C, LAM_V = 1.0, 0.1, 0.01, 10.0, 0.01
LOG2PI = float(np.log(2.0 * np.pi))

NT = 512
NNT = N // NT
import os
PHASES = os.environ.get("KPHASES", "enc,cnf,cham").split(",")
F32 = mybir.dt.float32
F16 = mybir.dt.float16
BF16 = mybir.dt.bfloat16
F8 = mybir.dt.float8e4
DR = mybir.MatmulPerfMode.DoubleRow
USE_F32R = True
MDT = mybir.dt.float32r if USE_F32R else F32

AF = mybir.ActivationFunctionType
ALU = mybir.AluOpType
AX = mybir.AxisListType


def _mm(ap):
    return ap


def host_precompute(w):
    f = np.float32
    W1, b1, W2, b2, W3, b3 = w["W1"], w["b1"], w["W2"], w["b2"], w["W3"], w["b3"]
    pre = {}
    pre["enc1"] = np.ascontiguousarray(
        np.concatenate([w["We1"].T, w["be1"][None, :]], 0), f)       # [3,256]
    pre["We2T"] = np.ascontiguousarray(
        w["We2"].T.reshape(2, 128, 256).transpose(1, 0, 2), f)        # [128,2,256]
    pre["be2r"] = np.ascontiguousarray(w["be2"][None, :], f)          # [1,256]
    pre["WmuT"] = np.ascontiguousarray(
        w["Wmu"].T.reshape(2, 128, 128).transpose(1, 0, 2), f)        # [128,2,128]
    pre["WlvT"] = np.ascontiguousarray(
        w["Wlv"].T.reshape(2, 128, 128).transpose(1, 0, 2), f)
    pre["bmulv"] = np.ascontiguousarray(
        np.stack([w["bmu"], w["blv"]], 1), f)                         # [128,2]

    W1p = W1[:, 0:2]
    W1t = W1[:, 2]
    pre["W1zT"] = np.ascontiguousarray(W1[:, 3:].T, f)                # [128,256]
    # a1top: W1p.T repeated for 20 evals x 2 m-blocks: [2, 20, 2, 128]
    pre["a1top"] = np.ascontiguousarray(
        np.broadcast_to(W1p.T.reshape(2, 1, 2, 128), (2, 20, 2, 128)), f)
    idx = np.arange(STEPS)
    pb3 = W1p @ b3
    TB_e = (idx * DT)[:, None] * W1t[None, :] + b1[None, :] \
        + (idx * DT)[:, None] * pb3[None, :]
    TB_g = (1.0 - idx * DT)[:, None] * W1t[None, :] + b1[None, :] \
        - (idx * DT)[:, None] * pb3[None, :]
    pre["TB"] = np.ascontiguousarray(np.concatenate([TB_e, TB_g], 0), f)  # [20,256]

    import ml_dtypes
    f8 = ml_dtypes.float8_e4m3
    pre["W2T8"] = np.ascontiguousarray(
        W2.T.reshape(2, 128, 256).transpose(1, 0, 2).astype(f8))      # [128,2,256]
    pre["b2r"] = np.ascontiguousarray(b2[None, :], f)                 # [1,256]
    pre["b2c"] = np.ascontiguousarray(b2.reshape(2, 128).T, f)        # [128,2]

    pre["vW"] = np.ascontiguousarray(
        (DT * W3.T).reshape(2, 128, 2).transpose(1, 0, 2), f)         # [128,2,2]
    pre["nvW"] = np.ascontiguousarray(
        (-DT * W3.T).reshape(2, 128, 2).transpose(1, 0, 2), f)

    c0, c1 = W1[:, 0], W1[:, 1]
    Wu = (W3[0][:, None] * W2) * c0[None, :] + (W3[1][:, None] * W2) * c1[None, :]
    pre["nWuT8"] = np.ascontiguousarray(
        (-Wu.T).reshape(2, 128, 256).transpose(1, 0, 2).astype(f8))   # [128,2,256]
    pre["rur"] = np.ascontiguousarray(Wu.sum(1)[None, :], f)          # [1,256]

    pre["ident2"] = np.eye(2, dtype=f)
    pre["identh"] = np.eye(128, dtype=np.float16)
    pre["b3c"] = np.ascontiguousarray(b3[:, None], f)                 # [2,1]
    pre["nb3c"] = np.ascontiguousarray(-b3[:, None], f)
    return pre


def _xs_rows(xb):
    """x-side fp16 hi/lo split rows for the chamfer D matmul.

    Pairs (by K row) with rsplit rows [rx_hi, ry_hi, rx_lo, ry_lo,
    rx_hi, ry_hi, one, one] so that
      D = r^2(bias) - 2 r.x + x^2  with only lo*lo terms (~1e-6) dropped.
    """
    xx = xb[:, 0].astype(np.float64)
    xy = xb[:, 1].astype(np.float64)
    m2x, m2y = -2.0 * xx, -2.0 * xy
    m2xh = m2x.astype(np.float16)
    m2xl = (m2x - m2xh.astype(np.float64)).astype(np.float16)
    m2yh = m2y.astype(np.float16)
    m2yl = (m2y - m2yh.astype(np.float64)).astype(np.float16)
    xsq = xx * xx + xy * xy
    xqh = xsq.astype(np.float16)
    xql = (xsq - xqh.astype(np.float64)).astype(np.float16)
    return np.ascontiguousarray(
        np.stack([m2xh, m2yh, m2xh, m2yh, m2xl, m2yl, xqh, xql]))


def build_in_maps(inputs):
    """Per-core input maps (shared weights + per-cloud tensors)."""
    pre = host_precompute(inputs)
    ones_row = np.ones((1, N), np.float32)
    in_maps = []
    for b in range(B):
        m = dict(pre)
        m["xT3"] = np.ascontiguousarray(
            np.concatenate([inputs["x"][b].T, ones_row], 0), np.float32)
        m["nT3"] = np.ascontiguousarray(
            np.concatenate([inputs["noise"][b].T, ones_row], 0), np.float32)
        m["epsc"] = np.ascontiguousarray(inputs["eps"][b][:, None], np.float32)
        m["xs"] = _xs_rows(inputs["x"][b])
        in_maps.append(m)
    return pre, in_maps


WEIGHT_SPECS = [
    ("enc1", (3, 256)), ("We2T", (128, 2, 256)), ("be2r", (1, 256)),
    ("WmuT", (128, 2, 128)), ("WlvT", (128, 2, 128)), ("bmulv", (128, 2)),
    ("W1zT", (128, 256)), ("a1top", (2, 20, 2, 128)), ("TB", (20, 256)),
    ("vW", (128, 2, 2)),
    ("nvW", (128, 2, 2)),
    ("b3c", (2, 1)), ("nb3c", (2, 1)), ("ident2", (2, 2)),
    ("b2c", (128, 2)),
]
F16_WEIGHTS = [("identh", (128, 128))]
F8_WEIGHTS = [("W2T8", (128, 2, 256)), ("nWuT8", (128, 2, 256))]


MM_NAMES = {"enc1", "We2T", "be2r", "W2T", "b2r", "vW", "nvW", "nWuT",
            "rur", "a1top"}


def build_nc(zero_b2=False):
    nc = bacc.Bacc("TRN2", target_bir_lowering=False, debug=False,
                   enable_asserts=False, num_devices=B)
    ins = {}
    ins["xT3"] = nc.dram_tensor("xT3", [3, N], F32, kind="ExternalInput").ap()
    ins["nT3"] = nc.dram_tensor("nT3", [3, N], F32, kind="ExternalInput").ap()
    ins["epsc"] = nc.dram_tensor("epsc", [LAT, 1], F32, kind="ExternalInput").ap()
    ins["xs"] = nc.dram_tensor("xs", [8, N], F16, kind="ExternalInput").ap()
    for name, shape in WEIGHT_SPECS:
        ins[name] = nc.dram_tensor(name, list(shape), F32, kind="ExternalInput").ap()
    for name, shape in F16_WEIGHTS:
        ins[name] = nc.dram_tensor(name, list(shape), F16, kind="ExternalInput").ap()
    for name, shape in F8_WEIGHTS:
        ins[name] = nc.dram_tensor(name, list(shape), F8,
                                   kind="ExternalInput").ap()
    outs = {}
    for name, shape in [("o_div", [128]), ("o_mu", [128]), ("o_lv", [128]),
                        ("o_chA", [128]), ("o_chB", [128]), ("o_sy2", [2]),
                        ("o_h2s", [128, 2])]:
        outs[name] = nc.dram_tensor(name, shape, F32, kind="ExternalOutput").ap()

    with tile.TileContext(nc) as tc:
        _body(nc, tc, ins, outs, zero_b2)
    nc.compile()
    return nc


def _body(nc, tc, ins, outs, zero_b2=False):
    from contextlib import ExitStack
    with ExitStack() as ctx:
        const = ctx.enter_context(tc.tile_pool(name="const", bufs=1))
        state = ctx.enter_context(tc.tile_pool(name="state", bufs=1))
        work = ctx.enter_context(tc.tile_pool(name="work", bufs=2))
        small = ctx.enter_context(tc.tile_pool(name="small", bufs=1))

        # ---- load constants ----
        # gpsimd (cast-DMA) queue order = need order: encoder tensors
        # first so the encoder can start, CNF weights behind the state
        # inits. Non-cast loads ride the sync/HWDGE queue in parallel.
        c = {}
        enc_first = ["enc1", "We2T", "be2r"]
        rest = [n for n, _ in WEIGHT_SPECS if n not in enc_first + ["a1top"]]
        shapes = dict(WEIGHT_SPECS)
        for name in enc_first + rest:
            shape = shapes[name]
            dt_ = MDT if name in MM_NAMES else F32
            c[name] = const.tile(list(shape), dt_, tag=name, name=f"c_{name}")
        for name, shape in F16_WEIGHTS + [("xs", (8, N))]:
            c[name] = const.tile(list(shape), F16, tag=name, name=f"c_{name}")
        for name, shape in F8_WEIGHTS:
            c[name] = const.tile(list(shape), F8, tag=name, name=f"c_{name}")
        ones_r = const.tile([1, N], MDT, tag="ones_r")
        xT3 = state.tile([4, N], MDT, tag="st3", bufs=3, name="xT3")
        nT3 = state.tile([3, N], MDT, tag="nT3", name="nT3")
        # Encoder weights: stage as F32 over HWDGE then engine-copy to
        # f32r (each gpsimd cast-DMA costs ~1us of descriptor-gen; the
        # DVE is idle here). xT3 stays on the cast queue but goes first.
        nc.gpsimd.dma_start(out=xT3[0:3, 0:N // 2], in_=ins["xT3"][:, 0:N // 2])
        nc.gpsimd.dma_start(out=xT3[0:3, N // 2:N], in_=ins["xT3"][:, N // 2:N])
        stg = {}
        for name in enc_first:
            stg[name] = const.tile(list(shapes[name]), F32, tag=f"stg_{name}",
                                   name=f"stg_{name}")
            nc.sync.dma_start(out=stg[name], in_=ins[name])
            nc.vector.tensor_tensor(c[name], stg[name], stg[name], ALU.min)
        nc.gpsimd.dma_start(out=nT3, in_=ins["nT3"])
        nc.sync.dma_start(out=ones_r, in_=xT3[2:3])
        for name in rest:
            eng = nc.gpsimd if name in MM_NAMES else nc.sync
            eng.dma_start(out=c[name], in_=ins[name])
        for name, _ in F16_WEIGHTS + [("xs", (8, N))] + F8_WEIGHTS:
            nc.sync.dma_start(out=c[name], in_=ins[name])
        # Double-buffered euler/gen state lives directly in the xT3/nT3
        # load tiles (step 0 reads them in place); the p=1 buffers only
        # need their ones row initialized -- 2 DMAs instead of 16.
        y1 = state.tile([3, N], MDT, tag="y1", name="y1")
        s1 = state.tile([3, N], MDT, tag="s1", name="s1")
        nc.sync.dma_start(out=y1[2:3], in_=xT3[2:3])
        nc.sync.dma_start(out=s1[2:3], in_=xT3[2:3])
        ybuf_t = [xT3, y1]
        sbuf_t = [nT3, s1]
        ybuf = [[t[0:3, nt * NT:(nt + 1) * NT] for nt in range(NNT)]
                for t in ybuf_t]
        sbuf_ = [[t[0:3, nt * NT:(nt + 1) * NT] for nt in range(NNT)]
                 for t in sbuf_t]
        eps_s = small.tile([LAT, 1], F32, tag="eps")
        nc.sync.dma_start(out=eps_s, in_=ins["epsc"])

        divacc = small.tile([128, 1], F32, tag="divacc")
        nc.vector.memset(divacc, 0.0)
        divslots = small.tile([128, STEPS * NNT], F32, tag="divslots")
        h2slots = small.tile([128, 2, STEPS * NNT], F32, tag="h2slots")

        # ================= encoder =================
        g_s = small.tile([128, 2], F32, tag="g")
        NTE = 1024
        gparts = small.tile([128, 2, N // NTE], F32, tag="gparts")
        with tc.tile_pool(name="psE", bufs=1, space="PSUM") as psE:
            for te in range(N // NTE):
                sl = slice(te * NTE, (te + 1) * NTE)
                a1e = psE.tile([128, 2 * NTE], F32, tag="enc1")
                for mb in range(2):
                    mbs = slice(mb * 128, (mb + 1) * 128)
                    for q in range(NTE // NT):
                        qs = slice(q * NT, (q + 1) * NT)
                        qg = slice(te * NTE + q * NT, te * NTE + (q + 1) * NT)
                        nc.tensor.matmul(a1e[:, mb * NTE:(mb + 1) * NTE][:, qs],
                                         _mm(c["enc1"][:, mbs]),
                                         _mm(xT3[0:3, qg]),
                                         start=True, stop=True)
                h1e = work.tile([128, 2 * NTE], MDT, tag="eh1")
                nc.scalar.activation(h1e, a1e, AF.Relu)
                a2e = psE.tile([128, 2 * NTE], F32, tag="enc2")
                for mb in range(2):
                    mbs = slice(mb * 128, (mb + 1) * 128)
                    om = a2e[:, mb * NTE:(mb + 1) * NTE]
                    for q in range(NTE // NT):
                        qs = slice(q * NT, (q + 1) * NT)
                        nc.tensor.matmul(om[:, qs], _mm(c["We2T"][:, 0, mbs]),
                                         _mm(h1e[:, 0:NTE][:, qs]),
                                         start=True, stop=False)
                        nc.tensor.matmul(om[:, qs], _mm(c["We2T"][:, 1, mbs]),
                                         _mm(h1e[:, NTE:2 * NTE][:, qs]),
                                         start=False, stop=False)
                        nc.tensor.matmul(om[:, qs], _mm(c["be2r"][:, mbs]),
                                         _mm(ones_r[:, 0:NT]), start=False,
                                         stop=True)
                h2e = work.tile([128, 2 * NTE], MDT, tag="eh2")
                nc.scalar.activation(h2e, a2e, AF.Relu)
                for mb in range(2):
                    nc.vector.tensor_reduce(gparts[:, mb, te:te + 1],
                                            h2e[:, mb * NTE:(mb + 1) * NTE],
                                            axis=AX.X, op=ALU.max)
            for mb in range(2):
                nc.vector.tensor_reduce(g_s[:, mb:mb + 1], gparts[:, mb, :],
                                        axis=AX.X, op=ALU.max)

        with tc.tile_pool(name="psM", bufs=1, space="PSUM") as psM:
            mu_ps = psM.tile([128, 1], F32, tag="mu", bufs=1)
            lv_ps = psM.tile([128, 1], F32, tag="lv", bufs=1)
            for kb in range(2):
                nc.tensor.matmul(mu_ps, _mm(c["WmuT"][:, kb, :]),
                                 _mm(g_s[:, kb:kb + 1]), start=(kb == 0),
                                 stop=(kb == 1))
                nc.tensor.matmul(lv_ps, _mm(c["WlvT"][:, kb, :]),
                                 _mm(g_s[:, kb:kb + 1]), start=(kb == 0),
                                 stop=(kb == 1))
            mu_s = small.tile([128, 1], F32, tag="mu_s")
            lv_s = small.tile([128, 1], F32, tag="lv_s")
            nc.vector.tensor_scalar(mu_s, mu_ps, c["bmulv"][:, 0:1], None, ALU.add)
            nc.vector.tensor_scalar(lv_s, lv_ps, c["bmulv"][:, 1:2], None, ALU.add)
            nc.sync.dma_start(out=outs["o_mu"], in_=mu_s)
            nc.sync.dma_start(out=outs["o_lv"], in_=lv_s)
            # z = mu + eps * exp(0.5*lv)
            e_s = small.tile([128, 1], F32, tag="e_s")
            nc.scalar.activation(e_s, lv_s, AF.Exp, scale=0.5)
            z_s = small.tile([128, 1], F32, tag="z_s")
            nc.vector.tensor_tensor(z_s, e_s, eps_s, ALU.mult)
            nc.vector.tensor_tensor(z_s, z_s, mu_s, ALU.add)
        with tc.tile_pool(name="psZ", bufs=1, space="PSUM") as psZ:
            # cz_row = z @ W1zT : [1, 256]; broadcast to 20 partitions
            # with a K=1 ones matmul (avoids a DRAM round-trip)
            cz_ps = psZ.tile([1, 256], F32, tag="cz", bufs=1)
            nc.tensor.matmul(cz_ps, _mm(z_s), _mm(c["W1zT"]), start=True, stop=True)
            czrow_s = small.tile([1, 256], MDT, tag="czrow_s")
            nc.vector.tensor_copy(czrow_s, cz_ps)
            czb_ps = psZ.tile([20, 256], F32, tag="czb", bufs=1)
            nc.tensor.matmul(czb_ps, _mm(ones_r[0:1, 0:20]), _mm(czrow_s),
                             start=True, stop=True)
            brows = state.tile([20, 256], MDT, tag="brows")
            nc.vector.tensor_tensor(brows, c["TB"], czb_ps, ALU.add)

        # a1w: [3, 20, 2, 128] K=3 stationary operands (W1p rows + bias row)
        a1w = state.tile([3, 20, 2, 128], MDT, tag="a1w")
        nc.gpsimd.dma_start(out=a1w[0:2], in_=ins["a1top"])
        nc.sync.dma_start(out=a1w[2:3].rearrange("a b c d -> a (b c d)"),
                            in_=brows)

        # ================= CNF: euler + gen =================
        if "cnf" not in PHASES:
            return
        # Issue order is software-pipelined: per (i, nt) iteration the PE
        # runs [a1E, a2E, a1G, a2G, vpsE, vpsG, ups] so the tail matmuls
        # (vps/ups, whose results are consumed off the critical path) fill
        # PE time while the next iteration's activations run; scr/ups sit
        # last so neither PE nor DVE head-of-line-blocks on the slow
        # Pool h1sq op.
        # Scheduler steering: the Act engine (tanh) is the CNF bottleneck,
        # so the chain feeding it (a1 -> h1 -> a2 -> h2, both branches)
        # gets iteration-local priorities while everything consumed off
        # the critical path (h1sq/h2sq/vps/state adds/ups/scr) is pushed
        # two iterations later -- the scheduler then prefers starting the
        # next iteration's matmuls over finishing this one's tail.
        pbase = tc.cur_priority
        with tc.tile_pool(name="psA", bufs=2, space="PSUM") as psA, \
                tc.tile_pool(name="psB", bufs=2, space="PSUM") as psB:
            for i in range(STEPS):
                for nt in range(NNT):
                    it = i * NNT + nt

                    def P(h, rank, tail=False):
                        h.ins.bass_priority = pbase + (it + (2 if tail else 0)) * 16 + rank
                    stE = ybuf[i % 2][nt]
                    stEn = ybuf[(i + 1) % 2][nt]
                    stG = sbuf_[i % 2][nt]
                    stGn = sbuf_[(i + 1) % 2][nt]
                    slot = it

                    # euler a1 -> h1 -> h1sq(pool)
                    a1E = psA.tile([128, 2 * NT], F32, tag="a1", bufs=1)
                    for mb in range(2):
                        P(nc.tensor.matmul(a1E[:, mb * NT:(mb + 1) * NT],
                                           _mm(a1w[:, i, mb, :]),
                                           _mm(stE[0:3, :]),
                                           start=True, stop=True), 0)
                    h1E = work.tile([128, 2, NT], F8, tag="h1")
                    P(nc.scalar.activation(
                        h1E.rearrange("p b n -> p (b n)"), a1E, AF.Tanh), 1)
                    h1sq = work.tile([128, 2, NT], F8, tag="h1sq")
                    P(nc.gpsimd.tensor_tensor(h1sq, h1E, h1E, ALU.mult), 8, True)
                    # euler a2 -> h2 (fp8 DoubleRow: K=256 in one matmul/mb)
                    a2E = psB.tile([128, 2 * NT], F32, tag="a2", bufs=2)
                    for mb in range(2):
                        mbs = slice(mb * 128, (mb + 1) * 128)
                        om = a2E[:, mb * NT:(mb + 1) * NT]
                        P(nc.tensor.matmul(om, _mm(c["W2T8"][:, :, mbs]),
                                           _mm(h1E), start=True, stop=True,
                                           perf_mode=DR), 2)
                    h2E = work.tile([128, 2 * NT], MDT, tag="h2")
                    for mb in range(2):
                        ms = slice(mb * NT, (mb + 1) * NT)
                        P(nc.scalar.activation(h2E[:, ms], a2E[:, ms], AF.Tanh,
                                               bias=c["b2c"][:, mb:mb + 1]), 3)
                    # gen a1 -> h1 -> a2 -> h2
                    a1G = psA.tile([128, 2 * NT], F32, tag="a1", bufs=1)
                    for mb in range(2):
                        P(nc.tensor.matmul(a1G[:, mb * NT:(mb + 1) * NT],
                                           _mm(a1w[:, STEPS + i, mb, :]),
                                           _mm(stG[0:3, :]),
                                           start=True, stop=True), 4)
                    h1G = work.tile([128, 2, NT], F8, tag="h1")
                    P(nc.scalar.activation(
                        h1G.rearrange("p b n -> p (b n)"), a1G, AF.Tanh), 5)
                    a2G = psB.tile([128, 2 * NT], F32, tag="a2", bufs=2)
                    for mb in range(2):
                        mbs = slice(mb * 128, (mb + 1) * 128)
                        om = a2G[:, mb * NT:(mb + 1) * NT]
                        P(nc.tensor.matmul(om, _mm(c["W2T8"][:, :, mbs]),
                                           _mm(h1G), start=True, stop=True,
                                           perf_mode=DR), 6)
                    h2G = work.tile([128, 2 * NT], MDT, tag="h2")
                    for mb in range(2):
                        ms = slice(mb * NT, (mb + 1) * NT)
                        P(nc.scalar.activation(h2G[:, ms], a2G[:, ms], AF.Tanh,
                                               bias=c["b2c"][:, mb:mb + 1]), 7)
                    # h2sq = h2*h2 with per-mb rowsum accumulation
                    # (feeds the ru * sum(s2) host-side correction)
                    h2sq = work.tile([128, 2 * NT], F32, tag="h2sq")
                    for mb in range(2):
                        ms = slice(mb * NT, (mb + 1) * NT)
                        P(nc.vector.scalar_tensor_tensor(
                            out=h2sq[:, ms], in0=h2E[:, ms], scalar=1.0,
                            in1=h2E[:, ms], op0=ALU.mult, op1=ALU.mult,
                            accum_out=h2slots[:, mb, slot:slot + 1]), 13, True)
                    # velocities + state updates
                    vpsE = psB.tile([2, NT], F32, tag="a2", bufs=2)
                    for kb in range(2):
                        P(nc.tensor.matmul(vpsE, _mm(c["vW"][:, kb, :]),
                                           _mm(h2E[:, kb * NT:(kb + 1) * NT]),
                                           start=(kb == 0), stop=(kb == 1)), 9, True)
                    P(nc.vector.tensor_tensor(stEn[0:2, :], stE[0:2, :],
                                              vpsE, ALU.add), 10, True)
                    vpsG = psB.tile([2, NT], F32, tag="a2", bufs=2)
                    for kb in range(2):
                        P(nc.tensor.matmul(vpsG, _mm(c["nvW"][:, kb, :]),
                                           _mm(h2G[:, kb * NT:(kb + 1) * NT]),
                                           start=(kb == 0), stop=(kb == 1)), 11, True)
                    P(nc.vector.tensor_tensor(stGn[0:2, :], stG[0:2, :],
                                              vpsG, ALU.add), 12, True)
                    # tail: divergence matmul (fp8 DoubleRow) + scr accum
                    ups = psA.tile([128, 2 * NT], F32, tag="ups", bufs=1)
                    for mb in range(2):
                        mbs = slice(mb * 128, (mb + 1) * 128)
                        om = ups[:, mb * NT:(mb + 1) * NT]
                        P(nc.tensor.matmul(om, _mm(c["nWuT8"][:, :, mbs]),
                                           _mm(h1sq), start=True, stop=True,
                                           perf_mode=DR), 14, True)
                    # scr = (h2sq - 1) * u ; divslots col = sum(scr)
                    # (negated s2*u -- host flips the sign)
                    scr = work.tile([128, 2 * NT], F32, tag="scr")
                    P(nc.vector.scalar_tensor_tensor(
                        out=scr, in0=h2sq, scalar=1.0, in1=ups,
                        op0=ALU.subtract, op1=ALU.mult,
                        accum_out=divslots[:, slot:slot + 1]), 15, True)
        tc.cur_priority = pbase + (STEPS * NNT + 3) * 16

        nc.vector.tensor_reduce(divacc, divslots, axis=AX.X, op=ALU.add)
        nc.sync.dma_start(out=outs["o_div"], in_=divacc)
        h2rows = small.tile([128, 2], F32, tag="h2rows")
        nc.vector.tensor_reduce(h2rows, h2slots, axis=AX.X, op=ALU.add)
        nc.sync.dma_start(out=outs["o_h2s"], in_=h2rows)

        # final y stats: y_true = y + b3 (per n-tile)
        sy2slots = small.tile([2, NNT], F32, tag="sy2slots")
        for nt in range(NNT):
            ytrue = work.tile([2, NT], F32, tag="yt", name="ytrue")
            nc.vector.tensor_scalar(ytrue, ybuf[STEPS % 2][nt][0:2, :],
                                    c["b3c"], None, ALU.add)
            sy2scr = work.tile([2, NT], F32, tag="scr2", name="sy2scr")
            nc.scalar.activation(sy2scr, ytrue, AF.Square,
                                 accum_out=sy2slots[:, nt:nt + 1])
        sy2 = small.tile([2, 1], F32, tag="sy2")
        nc.vector.tensor_reduce(sy2, sy2slots, axis=AX.X, op=ALU.add)
        nc.sync.dma_start(out=outs["o_sy2"], in_=sy2)

        # ================= chamfer =================
        if "cham" not in PHASES:
            return
        # D[p, m] = |r_p - x_m|^2 via a K=8 fp16 hi/lo split matmul
        # (1 cycle/row on PE vs fp32's 4) with the per-row r^2 term applied
        # as the bias of the PSUM->SBUF Relu copy on the scalar engine.
        # Accuracy: products of fp16 splits are exact in the fp32 PSUM
        # accumulate; only lo*lo cross terms (~1e-6 absolute) are dropped.
        # Min tracking runs on DVE in fp16/SBUF (4x mode).
        rr = work.tile([2, N], F32, tag="rr", bufs=1, name="rr")
        for nt in range(NNT):
            sl = slice(nt * NT, (nt + 1) * NT)
            nc.vector.tensor_scalar(rr[:, sl], sbuf_[STEPS % 2][nt][0:2, :],
                                    c["nb3c"], None, ALU.add)
        # rsqc[:, blk] = r^2 bias column per r-block via tiny PE transposes
        rsqc = small.tile([128, 16], F32, tag="rsqc")
        sq_scr = work.tile([128, 2], F32, tag="sqscr", bufs=2, name="sq_scr")
        with tc.tile_pool(name="psR", bufs=1, space="PSUM") as psR:
            rT = psR.tile([128, 32], F32, tag="rT")
            for k in range(16):
                nc.tensor.transpose(rT[:, 2 * k:2 * k + 2],
                                    rr[:, 128 * k:128 * (k + 1)], c["ident2"])
            for k in range(16):
                nc.scalar.activation(sq_scr, rT[:, 2 * k:2 * k + 2], AF.Square,
                                     accum_out=rsqc[:, k:k + 1])
        # r-side split rows: [rx_hi, ry_hi, rx_lo, ry_lo, rx_hi, ry_hi, 1, 1]
        # (hi/lo computed in partition-0-aligned tiles, assembled via DMA --
        # engine writes at partition offsets 2/4 are not allowed)
        rsplit = state.tile([8, N], F16, tag="rsplit", name="rsplit")
        nc.gpsimd.memset(rsplit, 1.0)
        r_hi = work.tile([2, N], F16, tag="r_hi", bufs=1, name="r_hi")
        r_lo = work.tile([2, N], F16, tag="r_lo", bufs=1, name="r_lo")
        nc.scalar.activation(r_hi, rr, AF.Copy)
        nc.vector.tensor_tensor(r_lo, rr, r_hi, ALU.subtract)
        nc.sync.dma_start(out=rsplit[0:2], in_=r_hi)
        nc.sync.dma_start(out=rsplit[2:4], in_=r_lo)
        nc.sync.dma_start(out=rsplit[4:6], in_=r_hi)

        chAmin = small.tile([128, 16], F32, tag="chAmin")
        runmin = state.tile([128, N], F16, tag="runmin")
        with tc.tile_pool(name="psD", bufs=2, space="PSUM") as psD:
            for blk in range(16):
                bsl = slice(blk * 128, (blk + 1) * 128)
                Dp = psD.tile([128, N], F32, tag="D")
                for mt in range(NNT):
                    msl = slice(mt * NT, (mt + 1) * NT)
                    nc.tensor.matmul(Dp[:, msl], rsplit[:, bsl],
                                     c["xs"][:, msl], start=True, stop=True)
                Df = work.tile([128, N], F16, tag="Df", bufs=2, name="Df")
                nc.scalar.activation(Df, Dp, AF.Relu,
                                     bias=rsqc[:, blk:blk + 1])
                # A-side min pyramid: two f16 2x TT-halvings before the
                # (always-1x) reduce sees only a quarter of the columns
                tmin = work.tile([128, N // 2], F16, tag="tmin", bufs=2,
                                 name="tmin")
                nc.vector.tensor_tensor(tmin, Df[:, 0:N // 2],
                                        Df[:, N // 2:N], ALU.min)
                tmin2 = work.tile([128, N // 4], F16, tag="tmin2", bufs=2,
                                  name="tmin2")
                nc.vector.tensor_tensor(tmin2, tmin[:, 0:N // 4],
                                        tmin[:, N // 4:N // 2], ALU.min)
                nc.vector.tensor_reduce(chAmin[:, blk:blk + 1], tmin2,
                                        axis=AX.X, op=ALU.min)
                if blk == 0:
                    nc.vector.tensor_copy(runmin, Df)
                else:
                    nc.vector.tensor_tensor(runmin, Df, runmin, ALU.min)
        # per-r side: sqrt, per-partition sums over the 16 blocks
        chs = small.tile([128, 16], F32, tag="chs")
        red = small.tile([128, 1], F32, tag="red")
        nc.scalar.activation(chs, chAmin, AF.Sqrt)
        nc.vector.tensor_reduce(red, chs, axis=AX.X, op=ALU.add)
        nc.sync.dma_start(out=outs["o_chA"], in_=red)
        # per-x side: PE-transpose runmin in 128x128 fp16 blocks, then
        # free-axis min reduces.
        chBmin = small.tile([128, 16], F32, tag="chBmin")
        with tc.tile_pool(name="psT", bufs=4, space="PSUM") as psT:
            for k in range(16):
                tp = psT.tile([128, 128], F16, tag="T")
                nc.tensor.transpose(tp, runmin[:, k * 128:(k + 1) * 128],
                                    c["identh"])
                nc.vector.tensor_reduce(chBmin[:, k:k + 1], tp,
                                        axis=AX.X, op=ALU.min)
        chs2 = small.tile([128, 16], F32, tag="chs2")
        red2 = small.tile([128, 1], F32, tag="red2")
        nc.scalar.activation(chs2, chBmin, AF.Sqrt)
        nc.vector.tensor_reduce(red2, chs2, axis=AX.X, op=ALU.add)
        nc.sync.dma_start(out=outs["o_chB"], in_=red2)


_NC_CACHE = {}


def _get_nc(zero_b2=False):
    key = ("nc", zero_b2)
    if key not in _NC_CACHE:
        _NC_CACHE[key] = build_nc(zero_b2)
    return _NC_CACHE[key]


def kernel(**inputs):
    inputs = {k: np.asarray(v, dtype=np.float32) if np.asarray(v).dtype != np.int32
              else np.asarray(v) for k, v in inputs.items()}
    nc = _get_nc()
    pre, in_maps = build_in_maps(inputs)
    res = run_bass_kernel_spmd(nc, in_maps, core_ids=list(range(B)))
    return combine(res.results, pre)


def combine(results, pre):
    ru2 = pre["rur"].reshape(2, 128).astype(np.float64)
    S_logpy = 0.0
    S_logdet = 0.0
    prior = 0.0
    entropy = 0.0
    chamA = 0.0
    chamB = 0.0
    for r in results:
        S_logpy += -0.5 * float(r["o_sy2"].sum()) - N * LOG2PI
        H = r["o_h2s"].astype(np.float64)  # [128, 2] = (partition, mb)
        corr = float((ru2 * (STEPS * N - H.T)).sum())
        S_logdet += DT * (corr - float(r["o_div"].sum()))
        mu = r["o_mu"].astype(np.float64)
        lv = r["o_lv"].astype(np.float64)
        prior += 0.5 * float((mu ** 2 + np.exp(lv) - lv - 1.0).sum())
        entropy += -0.5 * float((lv + 1.0 + LOG2PI).sum())
        chamA += float(r["o_chA"].sum())
        chamB += float(r["o_chB"].sum())
    recon = -(S_logpy + S_logdet) / (B * N)
    prior /= B
    entropy /= B
    cham = chamA / (B * N) + chamB / (B * N)
    vol = max(0.0, S_logdet / (B * N) - 10.0)
    return np.float32(LAM_R * recon + LAM_P * prior + LAM_E * entropy
                      + LAM_C * cham + LAM_V * vol)

